# revision 6
# baseline (speedup 1.0000x reference)
"""Overlapping-windows (conv1d-identity unfold) kernel for Trainium2.

out[b*T + t, w*C + c] = x[b, t + w - CTX, c]  (zero-padded in t), i.e. each
output row is a contiguous 494-element window of the zero-padded, flattened
per-batch [T + 2*CTX, C] array starting at t*C.

Strategy (pure DMA, no compute):
  - Shard batch dim across 8 cores (8 batches/core).
  - Per core, stage the padded input in SBUF as 128 partitions =
    8 batches x 16 time-chunks; partition (b, j) holds padded rows
    [j*K, j*K + K + 2*CTX) of batch b  (K = 125 rows, 3718 f32 / partition).
  - One SBUF->DRAM DMA writes the whole per-core output [16000, 494]:
    source AP reads overlapping windows (row stride C=26, length 494) within
    each partition; destination is fully contiguous.
"""

import numpy as np

N_CTX = 9
C = 26
W = 2 * N_CTX + 1          # 19
ROWLEN = W * C             # 494
B, T = 64, 2000
N_CORES = 8
B_C = B // N_CORES         # 8 batches per core
NCHUNK = 16                # time-chunks per batch -> 8*16 = 128 partitions
K = T // NCHUNK            # 125 rows per chunk
PF = (K + 2 * N_CTX) * C   # 3718 f32 per partition (chunk + halo)
EDGE = (K + N_CTX) * C     # 3484 f32 (edge chunks have one-sided halo)


def _build_nc():
    import concourse.bass as bass
    import concourse.mybir as mybir

    nc = bass.Bass(target_bir_lowering=False)
    x = nc.dram_tensor("x", [B_C, T, C], mybir.dt.float32, kind="ExternalInput")
    out = nc.dram_tensor(
        "out", [B_C * T, ROWLEN], mybir.dt.float32, kind="ExternalOutput"
    )

    with (
        nc.sbuf_tensor("xs", [128, PF], mybir.dt.float32) as xs,
        nc.semaphore("dma_sem") as dma_sem,
        nc.semaphore("msem") as msem,
        nc.Block() as block,
    ):

        @block.vector
        def _(vector):
            # Zero the halo strips that fall outside [0, T): head of chunk 0
            # and tail of chunk 15 for every batch. Full-tile memset is
            # simplest and still cheap (~4us).
            vector.memset(bass.AP(xs, 0, [[PF, 128], [1, PF]]), 0.0).then_inc(
                msem, 1
            )

        @block.sync
        def _(sync):
            sync.wait_ge(msem, 1)
            # chunk j=0 per batch: rows [0, K+CTX) land at offset CTX*C
            sync.dma_start(
                bass.AP(xs, N_CTX * C, [[NCHUNK * PF, B_C], [1, EDGE]]),
                bass.AP(x, 0, [[T * C, B_C], [1, EDGE]]),
            ).then_inc(dma_sem, 16)
            # chunks j=1..14 per batch: rows [j*K-CTX, j*K-CTX+143), full
            # lines. One DMA per batch — SBUF APs advance partitions only on
            # the first dim, so a second partition-crossing dim is illegal.
            for b in range(B_C):
                sync.dma_start(
                    bass.AP(
                        xs,
                        (b * NCHUNK + 1) * PF,
                        [[PF, NCHUNK - 2], [1, PF]],
                    ),
                    bass.AP(
                        x,
                        (b * T + K - N_CTX) * C,
                        [[K * C, NCHUNK - 2], [1, PF]],
                    ),
                ).then_inc(dma_sem, 16)
            # chunk j=15 per batch: rows [15*K-CTX, T) land at offset 0
            sync.dma_start(
                bass.AP(xs, (NCHUNK - 1) * PF, [[NCHUNK * PF, B_C], [1, EDGE]]),
                bass.AP(
                    x,
                    ((NCHUNK - 1) * K - N_CTX) * C,
                    [[T * C, B_C], [1, EDGE]],
                ),
            ).then_inc(dma_sem, 16)

            sync.wait_ge(dma_sem, 16 * (2 + B_C))
            # Output: partition p covers rows [p*K, (p+1)*K); row r reads
            # xs[p, r*C : r*C + ROWLEN] (overlapping strided reads), dst is
            # contiguous. Split across three descriptor generators (two
            # HWDGE rings + gpsimd SWDGE) so supply runs in parallel.
            ra, rb = 42, 84
            sync.dma_start(
                bass.AP(out, 0, [[K * ROWLEN, 128], [ROWLEN, ra], [1, ROWLEN]]),
                bass.AP(xs, 0, [[PF, 128], [C, ra], [1, ROWLEN]]),
            ).then_inc(dma_sem, 16)
            sync.wait_ge(dma_sem, 16 * (5 + B_C))

        @block.scalar
        def _(scalar):
            ra, rb = 42, 84
            scalar.wait_ge(dma_sem, 16 * (2 + B_C))
            scalar.dma_start(
                bass.AP(
                    out,
                    ra * ROWLEN,
                    [[K * ROWLEN, 128], [ROWLEN, rb - ra], [1, ROWLEN]],
                ),
                bass.AP(xs, ra * C, [[PF, 128], [C, rb - ra], [1, ROWLEN]]),
            ).then_inc(dma_sem, 16)
            scalar.wait_ge(dma_sem, 16 * (5 + B_C))

        @block.gpsimd
        def _(gpsimd):
            ra, rb = 42, 84
            gpsimd.wait_ge(dma_sem, 16 * (2 + B_C))
            gpsimd.dma_start(
                bass.AP(
                    out,
                    rb * ROWLEN,
                    [[K * ROWLEN, 128], [ROWLEN, K - rb], [1, ROWLEN]],
                ),
                bass.AP(xs, rb * C, [[PF, 128], [C, K - rb], [1, ROWLEN]]),
            ).then_inc(dma_sem, 16)
            gpsimd.wait_ge(dma_sem, 16 * (5 + B_C))

    return nc


def kernel(x: np.ndarray) -> np.ndarray:
    from concourse.bass_utils import run_bass_kernel_spmd

    x = np.ascontiguousarray(np.asarray(x), dtype=np.float32)
    assert x.shape == (B, T, C), x.shape

    nc = _build_nc()
    in_maps = [{"x": x[i * B_C : (i + 1) * B_C]} for i in range(N_CORES)]
    res = run_bass_kernel_spmd(nc, in_maps, core_ids=list(range(N_CORES)))
    return np.concatenate([r["out"] for r in res.results], axis=0)


# revision 16
# speedup vs baseline: 1.4665x; 1.4665x over previous
"""Overlapping-windows (conv1d-identity unfold) kernel for Trainium2.

out[b*T + t, w*C + c] = x[b, t + w - CTX, c]  (zero-padded in t), i.e. each
output row is a contiguous 494-element window of the zero-padded, flattened
per-batch [T + 2*CTX, C] array starting at t*C.

Strategy:
  - Shard batch dim across 8 cores (8 batches/core).
  - Per core, stage the padded input in SBUF as 128 partitions =
    8 batches x 16 time-chunks; partition (b, j) holds padded rows
    [j*K, j*K + K + 2*CTX) of batch b  (K = 125 rows, 3718 f32 / partition).
    Inbound DMAs are split across both HWDGE rings (sync + scalar).
  - 5 pipelined passes: DVE + ACT copy-unfold 25 output rows per partition
    into a triple-buffered tile ys[128, 25*494] (per-partition strided
    overlapping reads from xs), then one outbound DMA per pass writes
    6.3 MB with 49 KB contiguous descriptors (both sides contiguous per
    partition, ~430 GB/s). Big descriptors amortize the per-descriptor
    DMA-engine overhead that limits a direct 1976 B-descriptor store to
    ~250 GB/s; triple buffering keeps the outbound queue busy
    back-to-back.
"""

import numpy as np

N_CTX = 9
C = 26
W = 2 * N_CTX + 1          # 19
ROWLEN = W * C             # 494
B, T = 64, 2000
N_CORES = 8
B_C = B // N_CORES         # 8 batches per core
NCHUNK = 16                # time-chunks per batch -> 8*16 = 128 partitions
K = T // NCHUNK            # 125 rows per chunk
PF = (K + 2 * N_CTX) * C   # 3718 f32 per partition (chunk + halo)
EDGE = (K + N_CTX) * C     # 3484 f32 (edge chunks have one-sided halo)

NPASS = 5
NR = K // NPASS            # 25 output rows per partition per pass
VROWS = 11                 # rows unfolded by DVE per pass (ACT does the rest)
NBUF = 3                   # ys staging buffers
YF = NR * ROWLEN           # 12350 f32 per partition per staging buffer


def _build_nc():
    import concourse.bass as bass
    import concourse.mybir as mybir

    nc = bass.Bass(target_bir_lowering=False)
    x = nc.dram_tensor("x", [B_C, T, C], mybir.dt.float32, kind="ExternalInput")
    out = nc.dram_tensor(
        "out", [B_C * T, ROWLEN], mybir.dt.float32, kind="ExternalOutput"
    )

    with (
        nc.sbuf_tensor("xs", [128, PF], mybir.dt.float32) as xs,
        nc.sbuf_tensor("ys0", [128, YF], mybir.dt.float32) as ys0,
        nc.sbuf_tensor("ys1", [128, YF], mybir.dt.float32) as ys1,
        nc.sbuf_tensor("ys2", [128, YF], mybir.dt.float32) as ys2,
        nc.semaphore("in_sem") as in_sem,
        nc.semaphore("uv_sem") as uv_sem,
        nc.semaphore("ua_sem") as ua_sem,
        nc.semaphore("o_sem") as o_sem,
        nc.Block() as block,
    ):
        ys = [ys0, ys1, ys2]
        IN_DMAS = 4 + B_C  # target: 16 * IN_DMAS on in_sem

        zeros = nc.inline_tensor(
            np.zeros(B_C * N_CTX * C, dtype=np.float32), name="zstrip"
        )

        def inbound_mid(eng, b):
            # chunks j=1..14 of batch b: rows [j*K-CTX, j*K-CTX+143), full
            # lines. (SBUF APs advance partitions only on the first dim, so
            # one DMA per batch.)
            eng.dma_start(
                bass.AP(
                    xs, (b * NCHUNK + 1) * PF, [[PF, NCHUNK - 2], [1, PF]]
                ),
                bass.AP(
                    x,
                    (b * T + K - N_CTX) * C,
                    [[K * C, NCHUNK - 2], [1, PF]],
                ),
            ).then_inc(in_sem, 16)

        @block.sync
        def _(sync):
            # ---- inbound (shared with the scalar ring) ----
            # zero the out-of-range halo strips (head of chunk 0, tail of
            # chunk 15, per batch) from a NEFF-embedded zero tensor
            sync.dma_start(
                bass.AP(xs, 0, [[NCHUNK * PF, B_C], [1, N_CTX * C]]),
                bass.AP(zeros, 0, [[N_CTX * C, B_C], [1, N_CTX * C]]),
            ).then_inc(in_sem, 16)
            sync.dma_start(
                bass.AP(
                    xs,
                    (NCHUNK - 1) * PF + EDGE,
                    [[NCHUNK * PF, B_C], [1, N_CTX * C]],
                ),
                bass.AP(zeros, 0, [[N_CTX * C, B_C], [1, N_CTX * C]]),
            ).then_inc(in_sem, 16)
            # chunk j=0 per batch: rows [0, K+CTX) land at offset CTX*C
            sync.dma_start(
                bass.AP(xs, N_CTX * C, [[NCHUNK * PF, B_C], [1, EDGE]]),
                bass.AP(x, 0, [[T * C, B_C], [1, EDGE]]),
            ).then_inc(in_sem, 16)
            # chunk j=15 per batch: rows [15*K-CTX, T) land at offset 0
            sync.dma_start(
                bass.AP(xs, (NCHUNK - 1) * PF, [[NCHUNK * PF, B_C], [1, EDGE]]),
                bass.AP(
                    x,
                    ((NCHUNK - 1) * K - N_CTX) * C,
                    [[T * C, B_C], [1, EDGE]],
                ),
            ).then_inc(in_sem, 16)
            for b in range(3):
                inbound_mid(sync, b)

            # ---- outbound: one DMA per pass, 49 KB descriptors ----
            for m in range(NPASS):
                sync.wait_ge(uv_sem, m + 1)
                sync.wait_ge(ua_sem, m + 1)
                sync.dma_start(
                    bass.AP(
                        out,
                        m * NR * ROWLEN,
                        [[K * ROWLEN, 128], [1, YF]],
                    ),
                    bass.AP(ys[m % NBUF], 0, [[YF, 128], [1, YF]]),
                ).then_inc(o_sem, 16)
            sync.wait_ge(o_sem, 16 * NPASS)

        @block.vector
        def _(vector):
            # unfold rows [m*NR, m*NR+VROWS) of each partition
            for m in range(NPASS):
                vector.wait_ge(in_sem, 16 * IN_DMAS)
                if m >= NBUF:
                    vector.wait_ge(o_sem, 16 * (m - NBUF + 1))
                vector.tensor_copy(
                    bass.AP(
                        ys[m % NBUF],
                        0,
                        [[YF, 128], [ROWLEN, VROWS], [1, ROWLEN]],
                    ),
                    bass.AP(
                        xs,
                        m * NR * C,
                        [[PF, 128], [C, VROWS], [1, ROWLEN]],
                    ),
                ).then_inc(uv_sem, 1)

        @block.scalar
        def _(scalar):
            # the scalar (ACT) ring carries the other half of the inbound
            for b in range(3, B_C):
                inbound_mid(scalar, b)
            # unfold rows [m*NR+VROWS, (m+1)*NR) of each partition
            for m in range(NPASS):
                scalar.wait_ge(in_sem, 16 * IN_DMAS)
                if m >= NBUF:
                    scalar.wait_ge(o_sem, 16 * (m - NBUF + 1))
                scalar.copy(
                    bass.AP(
                        ys[m % NBUF],
                        VROWS * ROWLEN,
                        [[YF, 128], [ROWLEN, NR - VROWS], [1, ROWLEN]],
                    ),
                    bass.AP(
                        xs,
                        (m * NR + VROWS) * C,
                        [[PF, 128], [C, NR - VROWS], [1, ROWLEN]],
                    ),
                ).then_inc(ua_sem, 1)

    return nc


def kernel(x: np.ndarray) -> np.ndarray:
    from concourse.bass_utils import run_bass_kernel_spmd

    x = np.ascontiguousarray(np.asarray(x), dtype=np.float32)
    assert x.shape == (B, T, C), x.shape

    nc = _build_nc()
    in_maps = [{"x": x[i * B_C : (i + 1) * B_C]} for i in range(N_CORES)]
    res = run_bass_kernel_spmd(nc, in_maps, core_ids=list(range(N_CORES)))
    return np.concatenate([r["out"] for r in res.results], axis=0)
